# revision 61
# baseline (speedup 1.0000x reference)
"""ConvSTFT kernel for Trainium2 (Bass/Tile), data-parallel over batch on 8 cores.

Math: out[b, k, f, i] = sum_n xp[b, 320 f + n] * basis[i*513 + k, n]
where xp = x padded with 512 zeros on both sides, f in [0, 501), n in [0, 1024).

Layout: the host lays the padded signal out transposed (xst[b, p, j] =
xp[b, p + 64 j], pure data movement), so the device input load is a plain
contiguous DMA and contraction chunk c of frame f is the stride-5 column
view Xs[:, 2c + 5f].

The 1024-point windowed DFT is factored with a 2-level decimation-in-
frequency split (twiddles folded into the second-stage matrices), computed on
the vector engines as chunkwise butterflies:

    y  = w * frame          (window, per-partition scalars)
    s  = y[:512] + y[512:]      d = y[:512] - y[512:]
    ss = s[:256] + s[256:]      sd = s[:256] - s[256:]
    odd bins 2m+1 = DFT-rows(d),  bins 4t+2 = DFT-rows(sd),
    bins 4t (incl. 512) = DFT-rows(ss)

which cuts the tensor-engine work from 64 to 24 matmuls (502 cycles each)
per batch. All data is bf16 (fp32 PSUM accumulation); output is written bf16
and widened on the host — the 2e-2 error budget is ~3x above what this
costs. Bin 512 (re) rides in the zero imag-bin-0 column of the "4t" group
and is routed out by a tiny fixup.
"""

import numpy as np
from contextlib import ExitStack

import concourse.bass as bass
import concourse.tile as tile
from concourse import bacc, mybir

# problem constants (hardcoded per harness contract)
B, T = 32, 160000
NCORES = 8
BPC = B // NCORES          # batches per core
HOP, NFFT = 320, 1024
BINS, F = 513, 501         # freq bins, frames
FP = 502                   # frames padded to even
PAD = NFFT // 2            # 512
JC = 2560                  # Xs columns (>= 2*7 + 5*(FP-1) + 1, mult of 16)
L = 127 + 64 * (JC - 1) + 1  # 163904 padded xp length
BF16 = mybir.dt.bfloat16
FTW = 3072 + 16            # 24 lhs tiles + 8 fp32 window vectors (bitcast)

_STATE: dict = {}


def _build_nc():
    nc = bacc.Bacc(
        "TRN2", target_bir_lowering=False, debug=False, num_devices=NCORES
    )
    f32 = mybir.dt.float32
    add, sub, mult = (
        mybir.AluOpType.add, mybir.AluOpType.subtract, mybir.AluOpType.mult
    )
    xst = nc.dram_tensor("xst", [BPC, 128, JC], BF16, kind="ExternalInput").ap()
    ft = nc.dram_tensor("ft", [128, FTW], BF16, kind="ExternalInput").ap()
    out = nc.dram_tensor("out", [BPC, BINS, F, 2], BF16, kind="ExternalOutput").ap()

    with tile.TileContext(nc) as tc, ExitStack() as ctx:
        const_pool = ctx.enter_context(tc.tile_pool(name="const", bufs=1))
        xs_pool = ctx.enter_context(tc.tile_pool(name="xs", bufs=4))
        bf_pool = ctx.enter_context(tc.tile_pool(name="bf", bufs=4))
        st_pool = ctx.enter_context(tc.tile_pool(name="st", bufs=4))
        sx_pool = ctx.enter_context(tc.tile_pool(name="sx", bufs=2))
        acc_pool = ctx.enter_context(tc.tile_pool(name="acc", bufs=4, space="PSUM"))
        wu_pool = ctx.enter_context(tc.tile_pool(name="wu", bufs=1))

        # PE p-state warmup: keep the tensor engine's busy-streak alive
        # through the startup DMA window so real matmuls run at full clock
        dummy = wu_pool.tile([128, 512], BF16, tag="dummy")
        nc.vector.memset(dummy[:], 0)
        pw = acc_pool.tile([128, 1024], f32, tag="acc")
        for _ in range(14):
            nc.tensor.matmul(
                pw[:, 0:512], dummy[:, 0:128], dummy[:], start=True, stop=True
            )

        # startup DMAs: the tile framework serializes all DMAs into one
        # chain (~1.5-2us/hop); the scheduler orders it xs0, win, xs1, ft...
        # which lets batch-0 butterflies start ~4.7us in
        ft_sb = const_pool.tile([128, FTW], BF16, tag="ft")
        nc.scalar.dma_start(ft_sb[:, 3072:FTW], ft[:, 3072:FTW])

        # input loads: host has already laid x out transposed (xst[b, p, j]
        # = padded x[b, p + 64 j]), so these are plain contiguous DMAs
        xs_t, xs_i = [], []
        for b in range(BPC):
            xs = xs_pool.tile([128, JC], BF16, tag="xs")
            xs_t.append(xs)
            xs_i.append(nc.sync.dma_start(xs[:], xst[b]))

        nc.scalar.dma_start(ft_sb[:, 0:2048], ft[:, 0:2048])
        nc.scalar.dma_start(ft_sb[:, 2048:3072], ft[:, 2048:3072])

        f32w = lambda o: ft_sb[:, o:o + 2].bitcast(f32)
        wl = [f32w(3072 + 2 * c) for c in range(4)]
        wh = [f32w(3080 + 2 * c) for c in range(4)]

        # stx/stz buffers are zeroed once at startup; per-batch writes touch
        # only the even slots, the odd slots stay zero (imag of bins 0/512)
        stx_bufs, stz_bufs = [], []
        for _ in range(2):
            sx = sx_pool.tile([1, 2 * FP], BF16, tag="stx")
            nc.gpsimd.memset(sx[:], 0)
            stx_bufs.append(sx)
            sz = sx_pool.tile([1, 2 * FP], BF16, tag="stz")
            nc.gpsimd.memset(sz[:], 0)
            stz_bufs.append(sz)

        for b in range(BPC):
            xs = xs_t[b]

            def view(c):
                return xs[:, 2 * c: 2 * c + 5 * FP: 5]

            # butterflies, engine-balanced (STT has no DVE fast path, so use
            # tensor_scalar + tensor_tensor which run at 2x):
            #   u = wl*y_lo, t = wh*y_hi   (TS: DVE x7, Act x1)
            #   d = u - t, s = u + t       (TT: d + s23 on DVE, s01 on Pool)
            #   ss = s0+s2 (DVE), sd = s0-s2 (Pool)
            uv, tv = [], []
            for c in range(4):
                u_c = bf_pool.tile([128, FP], BF16, tag=f"u{c}")
                nc.vector.tensor_scalar_mul(u_c[:], view(c), wl[c])
                t_c = bf_pool.tile([128, FP], BF16, tag=f"t{c}")
                nc.vector.tensor_scalar_mul(t_c[:], view(c + 4), wh[c])
                uv.append(u_c); tv.append(t_c)
            dv, sv = [], [None] * 4
            for c in range(4):
                d_c = bf_pool.tile([128, FP], BF16, tag=f"d{c}")
                nc.vector.tensor_tensor(d_c[:], uv[c][:], tv[c][:], sub)
                dv.append(d_c)
            for c in range(4):
                s_c = bf_pool.tile([128, FP], BF16, tag=f"s{c}")
                eng = nc.gpsimd if c < 2 else nc.vector
                eng.tensor_tensor(s_c[:], uv[c][:], tv[c][:], add)
                sv[c] = s_c
            ssv, sdv = [], []
            for ch in range(2):
                ss_c = bf_pool.tile([128, FP], BF16, tag=f"ss{ch}")
                nc.vector.tensor_tensor(ss_c[:], sv[ch][:], sv[ch + 2][:], add)
                ssv.append(ss_c)
            for ch in range(2):
                sd_c = bf_pool.tile([128, FP], BF16, tag=f"sd{ch}")
                nc.gpsimd.tensor_tensor(sd_c[:], sv[ch][:], sv[ch + 2][:], sub)
                sdv.append(sd_c)

            # groups ordered by rhs readiness: d (DVE, early) -> ss (DVE) ->
            # sd (Pool, latest); (lhs base, rhs chunks, out row base, step).
            # The last batch ends on an odd group (no fixup ops in the tail).
            groups = [
                (0, dv, 1, 2, False),        # odd bins 1,3,..,255: m=0..127
                (1024, dv, 257, 2, False),   # odd bins 257,..,511: m=128..255
                (2560, ssv, 0, 4, True),     # bins 4t (+ bin 512 fixup)
                (2048, sdv, 2, 4, False),    # bins 4t+2
            ]
            for base, rhs_l, k0, kstep, is_ee in groups:
                # one 2-bank PSUM tile: re half at col 0, im half at col 512,
                # so a single multi-dim copy interleaves both into st
                ps = acc_pool.tile([128, 1024], f32, tag="acc")
                nch = len(rhs_l)
                for ci in range(nch):
                    lr = ft_sb[:, base + ci * 256: base + ci * 256 + 128]
                    li = ft_sb[:, base + ci * 256 + 128: base + ci * 256 + 256]
                    nc.tensor.matmul(
                        ps[:, 0:FP], lr, rhs_l[ci][:],
                        start=(ci == 0), stop=(ci == nch - 1)
                    )
                    nc.tensor.matmul(
                        ps[:, 512:512 + FP], li, rhs_l[ci][:],
                        start=(ci == 0), stop=(ci == nch - 1)
                    )
                if is_ee:
                    # partition 0 of the im half holds real bin 512, not the
                    # (identically zero) imag bin 0. Rows for bins 0 and 512
                    # go via the pre-zeroed stz/stx minibuffers so the main
                    # drain's DMA (rows 4..508) gates only on the drain.
                    st = st_pool.tile([128, 2 * FP], BF16, tag="st")
                    st3 = st[:].rearrange("p (f i) -> p f i", i=2)
                    ps3 = ps[:].rearrange("p (i f) -> p f i", i=2)[:, 0:FP, :]
                    nc.scalar.copy(st3, ps3)
                    stx = stx_bufs[b % 2]
                    stz = stz_bufs[b % 2]
                    if b == BPC - 1:
                        nc.vector.tensor_copy(stx[:, 0:2 * FP:2], ps[0:1, 512:512 + FP])
                        nc.vector.tensor_copy(stz[:, 0:2 * FP:2], ps[0:1, 0:FP])
                    else:
                        nc.scalar.copy(stx[:, 0:2 * FP:2], ps[0:1, 512:512 + FP])
                        nc.scalar.copy(stz[:, 0:2 * FP:2], ps[0:1, 0:FP])
                    dst_ee = bass.AP(
                        out.tensor, (b * BINS + k0 + kstep) * F * 2,
                        [[kstep * F * 2, 127], [1, F * 2]],
                    )
                    nc.sync.dma_start(dst_ee, st[1:128, 0:2 * F])
                    nc.sync.dma_start(out[b, 0:1, :, :], stz[:, 0:2 * F])
                    nc.sync.dma_start(out[b, 512:513, :, :], stx[:, 0:2 * F])
                    continue
                else:
                    st = st_pool.tile([128, 2 * FP], BF16, tag="st")
                    st3 = st[:].rearrange("p (f i) -> p f i", i=2)
                    ps3 = ps[:].rearrange("p (i f) -> p f i", i=2)[:, 0:FP, :]
                    if b == BPC - 1 and k0 == 2:
                        # very last group: halve the drain across DVE+Act and
                        # DMA each half as it lands to shorten the tail
                        HF = FP // 2
                        nc.vector.tensor_copy(st3[:, 0:HF, :], ps3[:, 0:HF, :])
                        nc.scalar.copy(st3[:, HF:FP, :], ps3[:, HF:FP, :])
                        dst1 = bass.AP(
                            out.tensor, (b * BINS + k0) * F * 2,
                            [[kstep * F * 2, 128], [1, 2 * HF]],
                        )
                        dst2 = bass.AP(
                            out.tensor, (b * BINS + k0) * F * 2 + 2 * HF,
                            [[kstep * F * 2, 128], [1, 2 * F - 2 * HF]],
                        )
                        nc.sync.dma_start(dst1, st[:, 0:2 * HF])
                        nc.scalar.dma_start(dst2, st[:, 2 * HF:2 * F])
                        continue
                    # final batch: split drains across DVE+Act so the tail
                    # chain is half as long (DVE is idle by then)
                    if b == BPC - 1 and k0 in (257,):
                        nc.vector.tensor_copy(st3, ps3)
                    else:
                        nc.scalar.copy(st3, ps3)
                dst = bass.AP(
                    out.tensor, (b * BINS + k0) * F * 2,
                    [[kstep * F * 2, 128], [1, F * 2]],
                )
                nc.sync.dma_start(dst, st[:, 0:2 * F])

    nc.compile()
    return nc


def _host_prep_basis(basis: np.ndarray):
    """Build the 2-level DIF lhs matrices + window columns, [128, FTW]."""
    w = np.asarray(basis, np.float64)[0]            # basis row 0 = window
    ftc = np.zeros((128, FTW), np.float64)
    p = np.arange(128)[:, None]
    j = np.arange(128)[None, :]

    def dft_block(qbase, kof, kstep):
        ang = 2.0 * np.pi * ((qbase + p) * (kof + kstep * j)) / NFFT
        return np.cos(ang), -np.sin(ang)

    for g in range(2):          # odd bins 2m+1, m = 128g + j, q = 128c + p
        for c in range(4):
            cosb, sinb = dft_block(128 * c, 2 * (128 * g) + 1, 2)
            ftc[:, g * 1024 + c * 256:g * 1024 + c * 256 + 128] = cosb
            ftc[:, g * 1024 + c * 256 + 128:g * 1024 + c * 256 + 256] = sinb
    for c in range(2):          # bins 4t+2, u = 128c + p
        cosb, sinb = dft_block(128 * c, 2, 4)
        ftc[:, 2048 + c * 256:2048 + c * 256 + 128] = cosb
        ftc[:, 2048 + c * 256 + 128:2048 + c * 256 + 256] = sinb
    for c in range(2):          # bins 4t, u = 128c + p; im col 0 -> bin512 re
        cosb, sinb = dft_block(128 * c, 0, 4)
        sinb = sinb.copy()
        sinb[:, 0] = np.cos(np.pi * (128 * c + np.arange(128)))   # (-1)^u
        ftc[:, 2560 + c * 256:2560 + c * 256 + 128] = cosb
        ftc[:, 2560 + c * 256 + 128:2560 + c * 256 + 256] = sinb
    return ftc, w


def _get_exec():
    """Build (once) and return a cached executor fn(in_maps) -> full output."""
    if "exec" in _STATE:
        return _STATE["exec"]

    from concourse import bass2jax

    nc = _build_nc()

    def run(in_maps):
        res = bass2jax.run_bass_via_pjrt(nc, in_maps, n_cores=NCORES)
        return np.concatenate(
            [np.asarray(r["out"]).astype(np.float32) for r in res], axis=0
        )

    _STATE["exec"] = run
    return run


def _prep_inputs(x: np.ndarray, basis: np.ndarray):
    import ml_dtypes

    xp_all = np.zeros((B, L), ml_dtypes.bfloat16)
    xp_all[:, PAD:PAD + T] = np.asarray(x, np.float32).astype(ml_dtypes.bfloat16)
    # transposed overlap layout: xst[b, p, j] = xp[b, p + 64 j]
    sb = xp_all.strides
    xst = np.ascontiguousarray(np.lib.stride_tricks.as_strided(
        xp_all, (B, 128, JC), (sb[0], sb[1], 64 * sb[1])))
    ftc, w = _host_prep_basis(basis)
    ft = ftc.astype(ml_dtypes.bfloat16)
    # window vectors as raw fp32 bits in the bf16 tensor (kernel bitcasts)
    w8 = np.stack(
        [w[128 * c:128 * c + 128] for c in range(4)]
        + [w[512 + 128 * c:512 + 128 * c + 128] for c in range(4)], axis=1
    ).astype(np.float32)                            # [128, 8]
    ft.view(np.uint16)[:, 3072:3088] = w8.view(np.uint16)
    in_maps = [
        {
            "xst": xst[BPC * c:BPC * (c + 1)],
            "ft": ft,
        }
        for c in range(NCORES)
    ]
    return in_maps


def kernel(x: np.ndarray, basis: np.ndarray) -> np.ndarray:
    run = _get_exec()
    in_maps = _prep_inputs(x, basis)
    return run(in_maps)                            # [32, 513, 501, 2]


# revision 66
# speedup vs baseline: 1.0118x; 1.0118x over previous
"""ConvSTFT kernel for Trainium2 (Bass/Tile), data-parallel over batch on 8 cores.

Math: out[b, k, f, i] = sum_n xp[b, 320 f + n] * basis[i*513 + k, n]
where xp = x padded with 512 zeros on both sides, f in [0, 501), n in [0, 1024).

Layout: the host lays the padded signal out transposed (xst[b, p, j] =
xp[b, p + 64 j], pure data movement), so the device input load is a plain
contiguous DMA and contraction chunk c of frame f is the stride-5 column
view Xs[:, 2c + 5f].

The 1024-point windowed DFT is factored with a 2-level decimation-in-
frequency split (twiddles folded into the second-stage matrices), computed on
the vector engines as chunkwise butterflies:

    y  = w * frame          (window, per-partition scalars)
    s  = y[:512] + y[512:]      d = y[:512] - y[512:]
    ss = s[:256] + s[256:]      sd = s[:256] - s[256:]
    odd bins 2m+1 = DFT-rows(d),  bins 4t+2 = DFT-rows(sd),
    bins 4t (incl. 512) = DFT-rows(ss)

which cuts the tensor-engine work from 64 to 24 matmuls (502 cycles each)
per batch. All data is bf16 (fp32 PSUM accumulation); output is written bf16
and widened on the host — the 2e-2 error budget is ~3x above what this
costs. Bin 512 (re) rides in the zero imag-bin-0 column of the "4t" group
and is routed out by a tiny fixup.
"""

import numpy as np
from contextlib import ExitStack

import concourse.bass as bass
import concourse.tile as tile
from concourse import bacc, mybir

# problem constants (hardcoded per harness contract)
B, T = 32, 160000
NCORES = 8
BPC = B // NCORES          # batches per core
HOP, NFFT = 320, 1024
BINS, F = 513, 501         # freq bins, frames
FP = 502                   # frames padded to even
PAD = NFFT // 2            # 512
JC = 2560                  # Xs columns (>= 2*7 + 5*(FP-1) + 1, mult of 16)
L = 127 + 64 * (JC - 1) + 1  # 163904 padded xp length
BF16 = mybir.dt.bfloat16
FTW = 3072 + 16            # 24 lhs tiles + 8 fp32 window vectors (bitcast)

_STATE: dict = {}


def _build_nc():
    nc = bacc.Bacc(
        "TRN2", target_bir_lowering=False, debug=False, num_devices=NCORES
    )
    f32 = mybir.dt.float32
    add, sub, mult = (
        mybir.AluOpType.add, mybir.AluOpType.subtract, mybir.AluOpType.mult
    )
    xst = nc.dram_tensor("xst", [BPC, 128, JC], BF16, kind="ExternalInput").ap()
    ft = nc.dram_tensor("ft", [128, FTW], BF16, kind="ExternalInput").ap()
    out = nc.dram_tensor("out", [BPC, BINS, F, 2], BF16, kind="ExternalOutput").ap()

    with tile.TileContext(nc) as tc, ExitStack() as ctx:
        const_pool = ctx.enter_context(tc.tile_pool(name="const", bufs=1))
        xs_pool = ctx.enter_context(tc.tile_pool(name="xs", bufs=4))
        bf_pool = ctx.enter_context(tc.tile_pool(name="bf", bufs=4))
        st_pool = ctx.enter_context(tc.tile_pool(name="st", bufs=4))
        sx_pool = ctx.enter_context(tc.tile_pool(name="sx", bufs=2))
        acc_pool = ctx.enter_context(tc.tile_pool(name="acc", bufs=4, space="PSUM"))
        wu_pool = ctx.enter_context(tc.tile_pool(name="wu", bufs=1))

        # PE p-state warmup: keep the tensor engine's busy-streak alive
        # through the startup DMA window so real matmuls run at full clock
        dummy = wu_pool.tile([128, 512], BF16, tag="dummy")
        nc.vector.memset(dummy[:], 0)
        pw = acc_pool.tile([128, 1024], f32, tag="acc")
        for _ in range(14):
            nc.tensor.matmul(
                pw[:, 0:512], dummy[:, 0:128], dummy[:], start=True, stop=True
            )

        # startup DMAs: the tile framework serializes all DMAs into one
        # chain (~1.5-2us/hop); the scheduler orders it xs0, win, xs1, ft...
        # which lets batch-0 butterflies start ~4.7us in
        ft_sb = const_pool.tile([128, FTW], BF16, tag="ft")
        nc.scalar.dma_start(ft_sb[:, 3072:FTW], ft[:, 3072:FTW])

        # input loads: host has already laid x out transposed (xst[b, p, j]
        # = padded x[b, p + 64 j]), so these are plain contiguous DMAs
        xs_t, xs_i = [], []
        for b in range(BPC):
            xs = xs_pool.tile([128, JC], BF16, tag="xs")
            xs_t.append(xs)
            xs_i.append(nc.sync.dma_start(xs[:], xst[b]))

        nc.scalar.dma_start(ft_sb[:, 0:2048], ft[:, 0:2048])
        nc.scalar.dma_start(ft_sb[:, 2048:3072], ft[:, 2048:3072])

        f32w = lambda o: ft_sb[:, o:o + 2].bitcast(f32)
        wl = [f32w(3072 + 2 * c) for c in range(4)]
        wh = [f32w(3080 + 2 * c) for c in range(4)]

        # stx/stz buffers are zeroed once at startup; per-batch writes touch
        # only the even slots, the odd slots stay zero (imag of bins 0/512)
        stx_bufs, stz_bufs = [], []
        for _ in range(2):
            sx = sx_pool.tile([1, 2 * FP], BF16, tag="stx")
            nc.gpsimd.memset(sx[:], 0)
            stx_bufs.append(sx)
            sz = sx_pool.tile([1, 2 * FP], BF16, tag="stz")
            nc.gpsimd.memset(sz[:], 0)
            stz_bufs.append(sz)

        for b in range(BPC):
            xs = xs_t[b]

            def view(c):
                return xs[:, 2 * c: 2 * c + 5 * FP: 5]

            # butterflies, engine-balanced (STT has no DVE fast path, so use
            # tensor_scalar + tensor_tensor which run at 2x):
            #   u = wl*y_lo, t = wh*y_hi   (TS: DVE x7, Act x1)
            #   d = u - t, s = u + t       (TT: d + s23 on DVE, s01 on Pool)
            #   ss = s0+s2 (DVE), sd = s0-s2 (Pool)
            uv, tv = [], []
            for c in range(4):
                u_c = bf_pool.tile([128, FP], BF16, tag=f"u{c}")
                nc.vector.tensor_scalar_mul(u_c[:], view(c), wl[c])
                t_c = bf_pool.tile([128, FP], BF16, tag=f"t{c}")
                nc.vector.tensor_scalar_mul(t_c[:], view(c + 4), wh[c])
                uv.append(u_c); tv.append(t_c)
            dv, sv = [], [None] * 4
            for c in range(4):
                d_c = bf_pool.tile([128, FP], BF16, tag=f"d{c}")
                nc.vector.tensor_tensor(d_c[:], uv[c][:], tv[c][:], sub)
                dv.append(d_c)
            for c in range(4):
                s_c = bf_pool.tile([128, FP], BF16, tag=f"s{c}")
                eng = nc.gpsimd if c < 2 else nc.vector
                eng.tensor_tensor(s_c[:], uv[c][:], tv[c][:], add)
                sv[c] = s_c
            ssv, sdv = [], []
            for ch in range(2):
                ss_c = bf_pool.tile([128, FP], BF16, tag=f"ss{ch}")
                nc.vector.tensor_tensor(ss_c[:], sv[ch][:], sv[ch + 2][:], add)
                ssv.append(ss_c)
            for ch in range(2):
                sd_c = bf_pool.tile([128, FP], BF16, tag=f"sd{ch}")
                nc.gpsimd.tensor_tensor(sd_c[:], sv[ch][:], sv[ch + 2][:], sub)
                sdv.append(sd_c)

            # groups ordered by rhs readiness: d (DVE, early) -> ss (DVE) ->
            # sd (Pool, latest); (lhs base, rhs chunks, out row base, step).
            # The last batch ends on an odd group (no fixup ops in the tail).
            groups = [
                (0, dv, 1, 2, False),        # odd bins 1,3,..,255: m=0..127
                (1024, dv, 257, 2, False),   # odd bins 257,..,511: m=128..255
                (2560, ssv, 0, 4, True),     # bins 4t (+ bin 512 fixup)
                (2048, sdv, 2, 4, False),    # bins 4t+2
            ]
            if b == BPC - 1:
                groups = [groups[0], groups[1], groups[3], groups[2]]
            for base, rhs_l, k0, kstep, is_ee in groups:
                # one 2-bank PSUM tile: re half at col 0, im half at col 512,
                # so a single multi-dim copy interleaves both into st
                ps = acc_pool.tile([128, 1024], f32, tag="acc")
                nch = len(rhs_l)
                for ci in range(nch):
                    lr = ft_sb[:, base + ci * 256: base + ci * 256 + 128]
                    li = ft_sb[:, base + ci * 256 + 128: base + ci * 256 + 256]
                    nc.tensor.matmul(
                        ps[:, 0:FP], lr, rhs_l[ci][:],
                        start=(ci == 0), stop=(ci == nch - 1)
                    )
                    nc.tensor.matmul(
                        ps[:, 512:512 + FP], li, rhs_l[ci][:],
                        start=(ci == 0), stop=(ci == nch - 1)
                    )
                if is_ee:
                    # partition 0 of the im half holds real bin 512, not the
                    # (identically zero) imag bin 0. Rows for bins 0 and 512
                    # go via the pre-zeroed stz/stx minibuffers so the main
                    # drain's DMA (rows 4..508) gates only on the drain.
                    st = st_pool.tile([128, 2 * FP], BF16, tag="st")
                    st3 = st[:].rearrange("p (f i) -> p f i", i=2)
                    ps3 = ps[:].rearrange("p (i f) -> p f i", i=2)[:, 0:FP, :]
                    nc.scalar.copy(st3, ps3)
                    stx = stx_bufs[b % 2]
                    stz = stz_bufs[b % 2]
                    if b == BPC - 1:
                        nc.vector.tensor_copy(stx[:, 0:2 * FP:2], ps[0:1, 512:512 + FP])
                        nc.vector.tensor_copy(stz[:, 0:2 * FP:2], ps[0:1, 0:FP])
                    else:
                        nc.scalar.copy(stx[:, 0:2 * FP:2], ps[0:1, 512:512 + FP])
                        nc.scalar.copy(stz[:, 0:2 * FP:2], ps[0:1, 0:FP])
                    dst_ee = bass.AP(
                        out.tensor, (b * BINS + k0 + kstep) * F * 2,
                        [[kstep * F * 2, 127], [1, F * 2]],
                    )
                    nc.sync.dma_start(dst_ee, st[1:128, 0:2 * F])
                    nc.sync.dma_start(out[b, 0:1, :, :], stz[:, 0:2 * F])
                    nc.sync.dma_start(out[b, 512:513, :, :], stx[:, 0:2 * F])
                    continue
                else:
                    st = st_pool.tile([128, 2 * FP], BF16, tag="st")
                    st3 = st[:].rearrange("p (f i) -> p f i", i=2)
                    ps3 = ps[:].rearrange("p (i f) -> p f i", i=2)[:, 0:FP, :]
                    if b == BPC - 1 and k0 == 2:
                        # very last group: halve the drain across DVE+Act and
                        # DMA each half as it lands to shorten the tail
                        HF = FP // 2
                        nc.vector.tensor_copy(st3[:, 0:HF, :], ps3[:, 0:HF, :])
                        nc.scalar.copy(st3[:, HF:FP, :], ps3[:, HF:FP, :])
                        dst1 = bass.AP(
                            out.tensor, (b * BINS + k0) * F * 2,
                            [[kstep * F * 2, 128], [1, 2 * HF]],
                        )
                        dst2 = bass.AP(
                            out.tensor, (b * BINS + k0) * F * 2 + 2 * HF,
                            [[kstep * F * 2, 128], [1, 2 * F - 2 * HF]],
                        )
                        nc.sync.dma_start(dst1, st[:, 0:2 * HF])
                        nc.scalar.dma_start(dst2, st[:, 2 * HF:2 * F])
                        continue
                    # final batch: split drains across DVE+Act so the tail
                    # chain is half as long (DVE is idle by then)
                    if b == BPC - 1 and k0 in (257,):
                        nc.vector.tensor_copy(st3, ps3)
                    else:
                        nc.scalar.copy(st3, ps3)
                dst = bass.AP(
                    out.tensor, (b * BINS + k0) * F * 2,
                    [[kstep * F * 2, 128], [1, F * 2]],
                )
                nc.sync.dma_start(dst, st[:, 0:2 * F])

    nc.compile()
    return nc


def _host_prep_basis(basis: np.ndarray):
    """Build the 2-level DIF lhs matrices + window columns, [128, FTW]."""
    w = np.asarray(basis, np.float64)[0]            # basis row 0 = window
    ftc = np.zeros((128, FTW), np.float64)
    p = np.arange(128)[:, None]
    j = np.arange(128)[None, :]

    def dft_block(qbase, kof, kstep):
        ang = 2.0 * np.pi * ((qbase + p) * (kof + kstep * j)) / NFFT
        return np.cos(ang), -np.sin(ang)

    for g in range(2):          # odd bins 2m+1, m = 128g + j, q = 128c + p
        for c in range(4):
            cosb, sinb = dft_block(128 * c, 2 * (128 * g) + 1, 2)
            ftc[:, g * 1024 + c * 256:g * 1024 + c * 256 + 128] = cosb
            ftc[:, g * 1024 + c * 256 + 128:g * 1024 + c * 256 + 256] = sinb
    for c in range(2):          # bins 4t+2, u = 128c + p
        cosb, sinb = dft_block(128 * c, 2, 4)
        ftc[:, 2048 + c * 256:2048 + c * 256 + 128] = cosb
        ftc[:, 2048 + c * 256 + 128:2048 + c * 256 + 256] = sinb
    for c in range(2):          # bins 4t, u = 128c + p; im col 0 -> bin512 re
        cosb, sinb = dft_block(128 * c, 0, 4)
        sinb = sinb.copy()
        sinb[:, 0] = np.cos(np.pi * (128 * c + np.arange(128)))   # (-1)^u
        ftc[:, 2560 + c * 256:2560 + c * 256 + 128] = cosb
        ftc[:, 2560 + c * 256 + 128:2560 + c * 256 + 256] = sinb
    return ftc, w


def _get_exec():
    """Build (once) and return a cached executor fn(in_maps) -> full output."""
    if "exec" in _STATE:
        return _STATE["exec"]

    from concourse import bass2jax

    nc = _build_nc()

    def run(in_maps):
        res = bass2jax.run_bass_via_pjrt(nc, in_maps, n_cores=NCORES)
        return np.concatenate(
            [np.asarray(r["out"]).astype(np.float32) for r in res], axis=0
        )

    _STATE["exec"] = run
    return run


def _prep_inputs(x: np.ndarray, basis: np.ndarray):
    import ml_dtypes

    xp_all = np.zeros((B, L), ml_dtypes.bfloat16)
    xp_all[:, PAD:PAD + T] = np.asarray(x, np.float32).astype(ml_dtypes.bfloat16)
    # transposed overlap layout: xst[b, p, j] = xp[b, p + 64 j]
    sb = xp_all.strides
    xst = np.ascontiguousarray(np.lib.stride_tricks.as_strided(
        xp_all, (B, 128, JC), (sb[0], sb[1], 64 * sb[1])))
    ftc, w = _host_prep_basis(basis)
    ft = ftc.astype(ml_dtypes.bfloat16)
    # window vectors as raw fp32 bits in the bf16 tensor (kernel bitcasts)
    w8 = np.stack(
        [w[128 * c:128 * c + 128] for c in range(4)]
        + [w[512 + 128 * c:512 + 128 * c + 128] for c in range(4)], axis=1
    ).astype(np.float32)                            # [128, 8]
    ft.view(np.uint16)[:, 3072:3088] = w8.view(np.uint16)
    in_maps = [
        {
            "xst": xst[BPC * c:BPC * (c + 1)],
            "ft": ft,
        }
        for c in range(NCORES)
    ]
    return in_maps


def kernel(x: np.ndarray, basis: np.ndarray) -> np.ndarray:
    run = _get_exec()
    in_maps = _prep_inputs(x, basis)
    return run(in_maps)                            # [32, 513, 501, 2]
